# revision 7
# baseline (speedup 1.0000x reference)
"""Trainium2 Bass kernel for IntervalClusterTriplet (hard-mining triplet loss).

Math: loss = mean_i relu(sqrt(max_{j in cluster(i)} d2_ij)
                       - sqrt(min_{j not in cluster(i)} d2_ij) + 1)
with d2_ij = n_i + n_j - 2 e_i.e_j. Only the max/min *values* are needed,
so no argmax/gather: per row we reduce (n_j - 2G_ij) and add n_i at the end.
The n_j term rides the matmul: after the Gram tile (-2G, K=128) a second
K=1 rank-1 matmul (ones^T @ n_row) accumulates n_j into the same PSUM
bank, so the vector engine does a single min-reduce per tile.

Sharding: rows of the distance matrix across 8 cores (1024 rows each).
Each core receives E^T rolled so its own 1024 columns come first, which
puts the same-cluster diagonal block at the same position on every core
(one SPMD program). Per-core output is the partial loss sum; host adds
the 8 scalars and divides by N.
"""

import numpy as np

import concourse.bacc as bacc
import concourse.mybir as mybir
import concourse.tile as tile
from concourse.bass_utils import run_bass_kernel_spmd

C, S, D = 1024, 8, 128
N = C * S              # 8192 embeddings
CORES = 8
M = N // CORES         # 1024 rows per core
P = 128                # partitions (rows per chunk)
CH = M // P            # 8 chunks per core
TN = 512               # column tile (one PSUM bank)
NT = N // TN           # 16 column tiles
BIG = 1.0e30
F32 = mybir.dt.float32
F32R = mybir.dt.float32r
ALU = mybir.AluOpType
AX = mybir.AxisListType
ACT = mybir.ActivationFunctionType

_CACHE: dict = {}


def build_program(reps: int = 1):
    """Build + compile the SPMD program. reps>1 wraps the body in a For_i
    loop (identical iterations) so wall-clock deltas isolate HW exec time."""
    nc = bacc.Bacc("TRN2", target_bir_lowering=False, debug=False)
    et_d = nc.dram_tensor("et", [D, N], F32R, kind="ExternalInput").ap()
    mmin_d = nc.dram_tensor("maskmin", [P, P], F32, kind="ExternalInput").ap()
    mmax_d = nc.dram_tensor("maskmax", [P, P], F32, kind="ExternalInput").ap()
    onesc_d = nc.dram_tensor("onesc", [P, 2], F32R, kind="ExternalInput").ap()
    onesr_d = nc.dram_tensor("onesr", [1, P], F32R, kind="ExternalInput").ap()
    out_d = nc.dram_tensor("out", [1, 1], F32, kind="ExternalOutput").ap()

    def body(tc, const, work, chunk, small, pbig, psmall):
        # ---- load inputs (8 column-chunk DMAs so they spread across queues)
        et = const.tile([D, N], F32R, tag="et")
        for c in range(CORES):
            nc.sync.dma_start(et[:, c * M:(c + 1) * M], et_d[:, c * M:(c + 1) * M])
        mmin = const.tile([P, P], F32, tag="mmin")
        nc.sync.dma_start(mmin, mmin_d)
        mmax = const.tile([P, P], F32, tag="mmax")
        nc.sync.dma_start(mmax, mmax_d)
        ones_c = const.tile([P, 2], F32R, tag="ones_c")
        nc.sync.dma_start(ones_c, onesc_d)
        ones_r = const.tile([1, P], F32R, tag="ones_r")
        nc.sync.dma_start(ones_r, onesr_d)

        # ---- setup: esq, em2, col norms n_row, my-row norms nmy
        esq = work.tile([D, N], F32R, tag="esq")
        for c in range(CORES):
            sl = slice(c * M, (c + 1) * M)
            nc.vector.tensor_mul(esq[:, sl], et[:, sl], et[:, sl])
        em2 = work.tile([D, M], F32R, tag="em2")
        nc.vector.tensor_scalar_mul(em2, et[:, 0:M], -2.0)

        n_row = work.tile([1, N], F32R, tag="n_row")  # n_j for all columns
        for t in range(NT):
            sl = slice(t * TN, (t + 1) * TN)
            pnr = psmall.tile([1, TN], F32, tag="pnr")
            nc.tensor.matmul(pnr, lhsT=ones_c[:, 0:1], rhs=esq[:, sl],
                             start=True, stop=True)
            nc.vector.tensor_copy(n_row[:, sl], pnr)

        nmy = work.tile([P, CH], F32, tag="nmy")  # my-row squared norms
        for m in range(CH):
            pm = psmall.tile([P, 2], F32, tag="pnr")
            nc.tensor.matmul(pm, lhsT=esq[:, m * P:(m + 1) * P],
                             rhs=ones_c, start=True, stop=True)
            nc.scalar.copy(nmy[:, m:m + 1], pm[:, 0:1])

        losses = work.tile([P, CH], F32, tag="losses")

        # ---- main loop: 8 chunks x 16 column tiles
        for m in range(CH):
            td, off = (m * P) // TN, (m * P) % TN
            mincols = chunk.tile([P, NT + 2], F32, tag="mincols")
            nc.vector.memset(mincols, 3.0e38)
            apm = small.tile([P, 1], F32, tag="apm")
            for t in range(NT):
                sl = slice(t * TN, (t + 1) * TN)
                pt = pbig.tile([P, TN], F32, tag="pt")
                nc.tensor.matmul(pt, lhsT=em2[:, m * P:(m + 1) * P],
                                 rhs=et[:, sl], start=True, stop=False)
                # rank-1 update: += 1 x n_row  (adds n_j to every row)
                nc.tensor.matmul(pt, lhsT=ones_r, rhs=n_row[:, sl],
                                 start=False, stop=True)
                if t != td:
                    nc.vector.tensor_reduce(mincols[:, t:t + 1], pt,
                                            axis=AX.X, op=ALU.min)
                else:
                    # diag block: masked max (hard positive) + masked min
                    scr = small.tile([P, P], F32, tag="scr")
                    nc.vector.tensor_add(scr, pt[:, off:off + P], mmax)
                    nc.vector.tensor_reduce(apm, scr, axis=AX.X, op=ALU.max)
                    scr2 = small.tile([P, P], F32, tag="scr2")
                    nc.vector.tensor_add(scr2, pt[:, off:off + P], mmin)
                    nc.vector.tensor_reduce(mincols[:, NT:NT + 1], scr2,
                                            axis=AX.X, op=ALU.min)
                    # min over the non-diag remainder of this tile
                    if off > 0:
                        nc.vector.tensor_reduce(mincols[:, t:t + 1],
                                                pt[:, 0:off], axis=AX.X, op=ALU.min)
                    if off + P < TN:
                        nc.vector.tensor_reduce(mincols[:, NT + 1:NT + 2],
                                                pt[:, off + P:TN], axis=AX.X, op=ALU.min)
            # epilogue: ap/an -> hinge loss for this chunk's 128 rows
            anm = small.tile([P, 1], F32, tag="anm")
            nc.vector.tensor_reduce(anm, mincols, axis=AX.X, op=ALU.min)
            apsq = small.tile([P, 1], F32, tag="apsq")
            nc.vector.tensor_scalar(apsq, apm, nmy[:, m:m + 1], 0.0,
                                    op0=ALU.add, op1=ALU.max)
            ansq = small.tile([P, 1], F32, tag="ansq")
            nc.vector.tensor_scalar(ansq, anm, nmy[:, m:m + 1], 0.0,
                                    op0=ALU.add, op1=ALU.max)
            ap = small.tile([P, 1], F32, tag="ap")
            nc.scalar.activation(ap, apsq, ACT.Sqrt)
            an = small.tile([P, 1], F32, tag="an")
            nc.scalar.activation(an, ansq, ACT.Sqrt)
            dmar = small.tile([P, 1], F32, tag="dmar")
            nc.vector.tensor_sub(dmar, ap, an)
            nc.scalar.activation(losses[:, m:m + 1], dmar, ACT.Relu, bias=1.0)

        # ---- final: sum over 8 chunks then over partitions
        lsum = work.tile([P, 1], F32R, tag="lsum")
        with nc.allow_low_precision(reason="f32r rounding of per-row loss is fine"):
            nc.vector.tensor_reduce(lsum, losses, axis=AX.X, op=ALU.add)
        ps = psmall.tile([1, 2], F32, tag="pnr")
        nc.tensor.matmul(ps, lhsT=lsum, rhs=ones_c, start=True, stop=True)
        outsb = work.tile([1, 1], F32, tag="outsb")
        nc.scalar.copy(outsb, ps[:, 0:1])
        nc.sync.dma_start(out_d, outsb)

    with tile.TileContext(nc) as tc:
        with (
            tc.tile_pool(name="const", bufs=1) as const,
            tc.tile_pool(name="work", bufs=1) as work,
            tc.tile_pool(name="chunk", bufs=2) as chunk,
            tc.tile_pool(name="small", bufs=3) as small,
            tc.tile_pool(name="pbig", bufs=6, space="PSUM") as pbig,
            tc.tile_pool(name="psmall", bufs=2, space="PSUM") as psmall,
        ):
            if reps == 1:
                body(tc, const, work, chunk, small, pbig, psmall)
            else:
                with tc.For_i(0, reps, 1):
                    body(tc, const, work, chunk, small, pbig, psmall)

    nc.compile()
    return nc


def make_in_maps(batch: np.ndarray):
    E = np.ascontiguousarray(batch.reshape(N, D).astype(np.float32, copy=False))
    ET = np.ascontiguousarray(E.T)
    idx = np.arange(P)
    same = (idx[:, None] // S) == (idx[None, :] // S)
    mmin = np.where(same, BIG, 0.0).astype(np.float32)
    mmax = np.where(same, 0.0, -BIG).astype(np.float32)
    in_maps = []
    for r in range(CORES):
        et_r = np.ascontiguousarray(np.roll(ET, -r * M, axis=1))
        in_maps.append({"et": et_r, "maskmin": mmin, "maskmax": mmax,
                        "onesc": np.ones((P, 2), np.float32),
                        "onesr": np.ones((1, P), np.float32)})
    return in_maps


def kernel(batch: np.ndarray) -> np.ndarray:
    if "nc" not in _CACHE:
        _CACHE["nc"] = build_program(reps=1)
    nc = _CACHE["nc"]
    in_maps = make_in_maps(np.asarray(batch))
    res = run_bass_kernel_spmd(nc, in_maps, core_ids=list(range(CORES)))
    total = sum(float(res.results[r]["out"][0, 0]) for r in range(CORES))
    return np.float32(total / N)


# revision 11
# speedup vs baseline: 10.8717x; 10.8717x over previous
"""Trainium2 Bass kernel for IntervalClusterTriplet (hard-mining triplet loss).

Math: loss = mean_i relu(sqrt(max_{j in cluster(i)} d2_ij)
                       - sqrt(min_{j not in cluster(i)} d2_ij) + 1)
with d2_ij = n_i + n_j - 2 e_i.e_j. Only the max/min *values* are needed,
so no argmax/gather: per row we reduce (n_j - 2G_ij) and add n_i at the end.
The n_j term rides the matmul: after the Gram tile (-2G, K=128) a second
K=1 rank-1 matmul (ones^T @ n_row) accumulates n_j into the same PSUM
bank, so the vector engine does a single min-reduce per tile.

Sharding: rows of the distance matrix across 8 cores (1024 rows each).
Each core receives E^T rolled so its own 1024 columns come first, which
puts the same-cluster diagonal block at the same position on every core
(one SPMD program). Per-core output is the partial loss sum; host adds
the 8 scalars and divides by N.
"""

import numpy as np

import concourse.bacc as bacc
import concourse.mybir as mybir
import concourse.tile as tile
from concourse.bass_utils import run_bass_kernel_spmd

C, S, D = 1024, 8, 128
N = C * S              # 8192 embeddings
CORES = 8
M = N // CORES         # 1024 rows per core
P = 128                # partitions (rows per chunk)
CH = M // P            # 8 chunks per core
TN = 512               # column tile (one PSUM bank)
NT = N // TN           # 16 column tiles
BIG = 1.0e30
F32 = mybir.dt.float32
F32R = mybir.dt.float32r
ALU = mybir.AluOpType
AX = mybir.AxisListType
ACT = mybir.ActivationFunctionType

_CACHE: dict = {}


def build_program(reps: int = 1, mode: str = "full"):
    """Build + compile the SPMD program. reps>1 wraps the body in a For_i
    loop (identical iterations) so wall-clock deltas isolate HW exec time.
    mode: 'full' | 'mm' (no DVE reduces) | 'mm_main' (also no rank-1) | 'dma'."""
    nc = bacc.Bacc("TRN2", target_bir_lowering=False, debug=False)
    et_d = nc.dram_tensor("et", [D, N], F32R, kind="ExternalInput").ap()
    mmin_d = nc.dram_tensor("maskmin", [P, P], F32, kind="ExternalInput").ap()
    mmax_d = nc.dram_tensor("maskmax", [P, P], F32, kind="ExternalInput").ap()
    onesc_d = nc.dram_tensor("onesc", [P, 2], F32R, kind="ExternalInput").ap()
    onesr_d = nc.dram_tensor("onesr", [1, P], F32R, kind="ExternalInput").ap()
    out_d = nc.dram_tensor("out", [1, 1], F32, kind="ExternalOutput").ap()

    def body(tc, const, work, chunk, small, pbig, psmall, mode="full"):
        # ---- load inputs (8 column-chunk DMAs so they spread across queues)
        et = const.tile([D, N], F32R, tag="et")
        for c in range(CORES):
            nc.sync.dma_start(et[:, c * M:(c + 1) * M], et_d[:, c * M:(c + 1) * M])
        mmin = const.tile([P, P], F32, tag="mmin")
        nc.sync.dma_start(mmin, mmin_d)
        mmax = const.tile([P, P], F32, tag="mmax")
        nc.sync.dma_start(mmax, mmax_d)
        ones_c = const.tile([P, 2], F32R, tag="ones_c")
        nc.sync.dma_start(ones_c, onesc_d)
        ones_r = const.tile([1, P], F32R, tag="ones_r")
        nc.sync.dma_start(ones_r, onesr_d)

        if mode == "dma":
            outsb = work.tile([1, 1], F32, tag="outsb")
            nc.scalar.copy(outsb, mmin[0:1, 0:1])
            nc.sync.dma_start(out_d, outsb)
            return

        # ---- setup: esq, em2, col norms n_row, my-row norms nmy
        esq = work.tile([D, N], F32R, tag="esq")
        for c in range(CORES):
            sl = slice(c * M, (c + 1) * M)
            nc.vector.tensor_mul(esq[:, sl], et[:, sl], et[:, sl])
        em2 = work.tile([D, M], F32R, tag="em2")
        nc.vector.tensor_scalar_mul(em2, et[:, 0:M], -2.0)

        n_row = work.tile([1, N], F32R, tag="n_row")  # n_j for all columns
        for t in range(NT):
            sl = slice(t * TN, (t + 1) * TN)
            pnr = psmall.tile([1, TN], F32, tag="pnr")
            nc.tensor.matmul(pnr, lhsT=ones_c[:, 0:1], rhs=esq[:, sl],
                             start=True, stop=True)
            nc.vector.tensor_copy(n_row[:, sl], pnr)

        nmy = work.tile([P, CH], F32, tag="nmy")  # my-row squared norms
        for m in range(CH):
            pm = psmall.tile([P, 2], F32, tag="pnr")
            nc.tensor.matmul(pm, lhsT=esq[:, m * P:(m + 1) * P],
                             rhs=ones_c, start=True, stop=True)
            nc.scalar.copy(nmy[:, m:m + 1], pm[:, 0:1])

        losses = work.tile([P, CH], F32, tag="losses")

        # ---- main loop: 8 chunks x 16 column tiles
        for m in range(CH):
            td, off = (m * P) // TN, (m * P) % TN
            mincols = chunk.tile([P, NT + 2], F32, tag="mincols")
            nc.vector.memset(mincols, 3.0e38)
            apm = small.tile([P, 1], F32, tag="apm")
            for t in range(NT):
                sl = slice(t * TN, (t + 1) * TN)
                pt = pbig.tile([P, TN], F32, tag="pt")
                main_stop = (mode == "mm_main")
                if mode != "red":
                    nc.tensor.matmul(pt, lhsT=em2[:, m * P:(m + 1) * P],
                                     rhs=et[:, sl], start=True, stop=main_stop)
                    # rank-1 update: += 1 x n_row  (adds n_j to every row)
                    if mode != "mm_main":
                        nc.tensor.matmul(pt, lhsT=ones_r, rhs=n_row[:, sl],
                                         start=False, stop=True)
                if mode in ("mm", "mm_main"):
                    continue
                if t != td:
                    nc.vector.tensor_reduce(mincols[:, t:t + 1], pt,
                                            axis=AX.X, op=ALU.min)
                else:
                    # diag block: masked max (hard positive) + masked min
                    scr = small.tile([P, P], F32, tag="scr")
                    nc.vector.tensor_add(scr, pt[:, off:off + P], mmax)
                    nc.vector.tensor_reduce(apm, scr, axis=AX.X, op=ALU.max)
                    scr2 = small.tile([P, P], F32, tag="scr2")
                    nc.vector.tensor_add(scr2, pt[:, off:off + P], mmin)
                    nc.vector.tensor_reduce(mincols[:, NT:NT + 1], scr2,
                                            axis=AX.X, op=ALU.min)
                    # min over the non-diag remainder of this tile
                    if off > 0:
                        nc.vector.tensor_reduce(mincols[:, t:t + 1],
                                                pt[:, 0:off], axis=AX.X, op=ALU.min)
                    if off + P < TN:
                        nc.vector.tensor_reduce(mincols[:, NT + 1:NT + 2],
                                                pt[:, off + P:TN], axis=AX.X, op=ALU.min)
            if mode in ("mm", "mm_main"):
                nc.vector.memset(losses[:, m:m + 1], 0.0)
                continue
            # epilogue: ap/an -> hinge loss for this chunk's 128 rows
            anm = small.tile([P, 1], F32, tag="anm")
            nc.vector.tensor_reduce(anm, mincols, axis=AX.X, op=ALU.min)
            apsq = small.tile([P, 1], F32, tag="apsq")
            nc.vector.tensor_scalar(apsq, apm, nmy[:, m:m + 1], 0.0,
                                    op0=ALU.add, op1=ALU.max)
            ansq = small.tile([P, 1], F32, tag="ansq")
            nc.vector.tensor_scalar(ansq, anm, nmy[:, m:m + 1], 0.0,
                                    op0=ALU.add, op1=ALU.max)
            ap = small.tile([P, 1], F32, tag="ap")
            nc.scalar.activation(ap, apsq, ACT.Sqrt)
            an = small.tile([P, 1], F32, tag="an")
            nc.scalar.activation(an, ansq, ACT.Sqrt)
            dmar = small.tile([P, 1], F32, tag="dmar")
            nc.vector.tensor_sub(dmar, ap, an)
            nc.scalar.activation(losses[:, m:m + 1], dmar, ACT.Relu, bias=1.0)

        # ---- final: sum over 8 chunks then over partitions
        lsum = work.tile([P, 1], F32R, tag="lsum")
        with nc.allow_low_precision(reason="f32r rounding of per-row loss is fine"):
            nc.vector.tensor_reduce(lsum, losses, axis=AX.X, op=ALU.add)
        ps = psmall.tile([1, 2], F32, tag="pnr")
        nc.tensor.matmul(ps, lhsT=lsum, rhs=ones_c, start=True, stop=True)
        outsb = work.tile([1, 1], F32, tag="outsb")
        nc.scalar.copy(outsb, ps[:, 0:1])
        nc.sync.dma_start(out_d, outsb)

    with tile.TileContext(nc) as tc:
        with (
            tc.tile_pool(name="const", bufs=1) as const,
            tc.tile_pool(name="work", bufs=1) as work,
            tc.tile_pool(name="chunk", bufs=2) as chunk,
            tc.tile_pool(name="small", bufs=3) as small,
            tc.tile_pool(name="pbig", bufs=7, space="PSUM") as pbig,
            tc.tile_pool(name="psmall", bufs=1, space="PSUM") as psmall,
        ):
            if reps == 1:
                body(tc, const, work, chunk, small, pbig, psmall, mode)
            else:
                with tc.For_i(0, reps, 1):
                    body(tc, const, work, chunk, small, pbig, psmall, mode)

    nc.compile()
    return nc


def make_in_maps(batch: np.ndarray):
    E = np.ascontiguousarray(batch.reshape(N, D).astype(np.float32, copy=False))
    ET = np.ascontiguousarray(E.T)
    idx = np.arange(P)
    same = (idx[:, None] // S) == (idx[None, :] // S)
    mmin = np.where(same, BIG, 0.0).astype(np.float32)
    mmax = np.where(same, 0.0, -BIG).astype(np.float32)
    in_maps = []
    for r in range(CORES):
        et_r = np.ascontiguousarray(np.roll(ET, -r * M, axis=1))
        in_maps.append({"et": et_r, "maskmin": mmin, "maskmax": mmax,
                        "onesc": np.ones((P, 2), np.float32),
                        "onesr": np.ones((1, P), np.float32)})
    return in_maps


def kernel(batch: np.ndarray) -> np.ndarray:
    if "nc" not in _CACHE:
        _CACHE["nc"] = build_program(reps=1)
    nc = _CACHE["nc"]
    in_maps = make_in_maps(np.asarray(batch))
    res = run_bass_kernel_spmd(nc, in_maps, core_ids=list(range(CORES)))
    total = sum(float(res.results[r]["out"][0, 0]) for r in range(CORES))
    return np.float32(total / N)
